# revision 10
# baseline (speedup 1.0000x reference)
"""HGNN (DGL-style hypergraph conv x3) Bass kernel for trn2, 8 NeuronCores.

Math (per layer, weights/bias W,b):
    out = (D_v^-1 B^T D_e^-1 B X) @ W + b         (+ relu / final log_softmax)
where B is the (edge x node) incidence matrix given by (node_idx, edge_idx)
pairs. W commutes past the (linear, row-wise-scaled) aggregations. Layers 1-2
apply W after aggregation (width stays 256); for layer 3 we apply W3 FIRST
(h3 = H2 @ W3, width 40) so the last layer's aggregations move 6.4x less data.

Sharding: edges / nodes are 1-D range-partitioned across the 8 cores; the
incidence nnz are assigned to the core owning the edge (edge-side pass) /
the node (node-side pass). Feature tables are replicated via AllGather so row
gathers are always local. All tables / gathered rows / matmul operands are
bf16 (PSUM accumulation stays fp32); X and the weights are cast host-side.

Segment sums run on the tensor engine: for each 128-nnz tile of the sorted
incidence stream, a 0/1 selection matrix S (built on the vector engine by
comparing per-nnz local segment ids against an iota row) maps gathered rows
into a PSUM accumulator indexed by segment within a 128-wide block. On the
node side the matmul runs "transposed" (G^T @ S) so the per-block result is
already laid out [feat, node] and feeds the W matmul without PE transposes.
Padding slots carry segment id -1 and contribute nothing.
"""
import hashlib
import os
import sys

import numpy as np

# A wedged NeuronCore (from a prior crashed/raced run) silently corrupts
# indirect-DMA gathers; resetting cores at runtime init restores clean state.
os.environ.setdefault("NEURON_RT_RESET_CORES", "1")

sys.path.insert(0, "/opt/trn_rl_repo")

import ml_dtypes

BF16 = ml_dtypes.bfloat16

V, E, NNZ = 50000, 20000, 500000
D = 256
F3 = 40                    # final layer width (W3 applied before aggregation)
NCORES = 8
EPC = E // NCORES          # 2500 edges per core
VPC = V // NCORES          # 6250 nodes per core
NBE = (EPC + 127) // 128   # 20 edge blocks per core
NBV = (VPC + 127) // 128   # 49 node blocks per core
TC = 8                     # 128-nnz tiles per gather chunk

P = 128


def _side_arrays(seg_local, other_idx, n_blocks, TB):
    """Build [128, n_tiles] gather-index / local-segment-id arrays for one
    core's sorted nnz stream (sorted by seg_local). TB[b] = padded tile count
    for block b (common across cores)."""
    n_tiles = sum(TB)
    idx = np.zeros((P, n_tiles), dtype=np.int32)
    luc = np.full((P, n_tiles), -1.0, dtype=BF16)
    counts = np.bincount(seg_local // P, minlength=n_blocks)
    offs = np.concatenate([[0], np.cumsum(counts)])
    col = 0
    for b in range(n_blocks):
        lo, hi = offs[b], offs[b + 1]
        s = np.arange(hi - lo)
        t, p = s // P, s % P
        idx[p, col + t] = other_idx[lo:hi]
        luc[p, col + t] = (seg_local[lo:hi] - P * b).astype(np.float32)
        col += TB[b]
    return idx, luc


def _preprocess(node_idx, edge_idx):
    ni = np.asarray(node_idx, dtype=np.int64)
    ei = np.asarray(edge_idx, dtype=np.int64)
    deg_e = np.bincount(ei, minlength=E)
    deg_v = np.bincount(ni, minlength=V)
    rde_full = (1.0 / np.maximum(deg_e, 1)).astype(np.float32)
    rdv_full = (1.0 / np.maximum(deg_v, 1)).astype(np.float32)

    # ---- edge-side: nnz grouped by owning edge range, sorted by edge
    e_sorted = []
    for c in range(NCORES):
        sel = (ei >= c * EPC) & (ei < (c + 1) * EPC)
        el = ei[sel] - c * EPC
        nn = ni[sel]
        order = np.argsort(el, kind="stable")
        e_sorted.append((el[order], nn[order]))
    TBe = [0] * NBE
    for c in range(NCORES):
        cnt = np.bincount(e_sorted[c][0] // P, minlength=NBE)
        for b in range(NBE):
            TBe[b] = max(TBe[b], -(-int(cnt[b]) // P))
    # pad total to a multiple of TC by extending the last block
    TE = sum(TBe)
    TBe[-1] += (-TE) % TC
    TE = sum(TBe)

    # ---- node-side: nnz grouped by owning node range, sorted by node
    v_sorted = []
    for c in range(NCORES):
        sel = (ni >= c * VPC) & (ni < (c + 1) * VPC)
        vl = ni[sel] - c * VPC
        ee = ei[sel]
        order = np.argsort(vl, kind="stable")
        v_sorted.append((vl[order], ee[order]))
    TBv = [0] * NBV
    for c in range(NCORES):
        cnt = np.bincount(v_sorted[c][0] // P, minlength=NBV)
        for b in range(NBV):
            TBv[b] = max(TBv[b], -(-int(cnt[b]) // P))
    TV = sum(TBv)
    TBv[-1] += (-TV) % TC
    TV = sum(TBv)

    per_core = []
    for c in range(NCORES):
        idxe, luce = _side_arrays(e_sorted[c][0], e_sorted[c][1], NBE, TBe)
        idxv, lucv = _side_arrays(v_sorted[c][0], v_sorted[c][1], NBV, TBv)
        rde = np.ones((P, NBE), dtype=np.float32)
        for b in range(NBE):
            n = min(P, EPC - P * b)
            rde[:n, b] = rde_full[c * EPC + P * b: c * EPC + P * b + n]
        rdv = np.ones((P, NBV), dtype=np.float32)
        for b in range(NBV):
            n = min(P, VPC - P * b)
            rdv[:n, b] = rdv_full[c * VPC + P * b: c * VPC + P * b + n]
        per_core.append(dict(idxe=idxe, luce=luce, idxv=idxv, lucv=lucv,
                             rde=rde, rdv=rdv))
    return dict(TBe=TBe, TBv=TBv, TE=TE, TV=TV, per_core=per_core)


def _flatten_blocks(TB):
    """[(block, is_first, is_last)] per tile."""
    out = []
    for b, T in enumerate(TB):
        for t in range(T):
            out.append((b, t == 0, t == T - 1))
    return out


def _build(meta, debug=None):
    """debug: None = full kernel; 'e0' = dump eloc0 (layer-0 edge agg);
    'v0' = dump vloc0 (after layer 0); 'h3' = dump h3loc (after layer 1);
    'e2' = dump eloc2 (layer-2 edge agg)."""
    import concourse.bacc as bacc
    import concourse.bass as bass
    import concourse.mybir as mybir
    import concourse.tile as tile

    f32 = mybir.dt.float32
    bf16 = mybir.dt.bfloat16
    i32 = mybir.dt.int32
    TE, TV = meta["TE"], meta["TV"]
    tiles_e = _flatten_blocks(meta["TBe"])
    tiles_v = _flatten_blocks(meta["TBv"])

    nc = bacc.Bacc("TRN2", target_bir_lowering=False, debug=False,
                   num_devices=NCORES)

    xt = nc.dram_tensor("xt", [V, D], bf16, kind="ExternalInput")
    idxe_d = nc.dram_tensor("idxe", [P, TE], i32, kind="ExternalInput")
    luce_d = nc.dram_tensor("luce", [P, TE], bf16, kind="ExternalInput")
    idxv_d = nc.dram_tensor("idxv", [P, TV], i32, kind="ExternalInput")
    lucv_d = nc.dram_tensor("lucv", [P, TV], bf16, kind="ExternalInput")
    rde_d = nc.dram_tensor("rde", [P, NBE], f32, kind="ExternalInput")
    rdv_d = nc.dram_tensor("rdv", [P, NBV], f32, kind="ExternalInput")
    w1_d = nc.dram_tensor("w1", [D, D], bf16, kind="ExternalInput")
    w2_d = nc.dram_tensor("w2", [D, D], bf16, kind="ExternalInput")
    w3_d = nc.dram_tensor("w3", [D, F3], bf16, kind="ExternalInput")
    b1_d = nc.dram_tensor("b1x", [P, D], f32, kind="ExternalInput")
    b2_d = nc.dram_tensor("b2x", [P, D], f32, kind="ExternalInput")
    b3_d = nc.dram_tensor("b3x", [P, F3], f32, kind="ExternalInput")
    iota_d = nc.dram_tensor("iota", [P, P], bf16, kind="ExternalInput")
    ident_d = nc.dram_tensor("ident", [P, P], bf16, kind="ExternalInput")
    if debug == "e0":
        out_d = nc.dram_tensor("out", [EPC, D], f32, kind="ExternalOutput")
    elif debug == "v0":
        out_d = nc.dram_tensor("out", [VPC, D], f32, kind="ExternalOutput")
    elif debug == "h3":
        out_d = nc.dram_tensor("out", [VPC, F3], f32, kind="ExternalOutput")
    elif debug == "e2":
        out_d = nc.dram_tensor("out", [EPC, F3], f32, kind="ExternalOutput")
    else:
        out_d = nc.dram_tensor("out", [VPC, F3], f32, kind="ExternalOutput")

    eloc = [nc.dram_tensor("eloc0", [EPC, D], bf16),
            nc.dram_tensor("eloc1", [EPC, D], bf16),
            nc.dram_tensor("eloc2", [EPC, F3], bf16)]
    etab = [nc.dram_tensor("etab0", [E, D], bf16),
            nc.dram_tensor("etab1", [E, D], bf16),
            nc.dram_tensor("etab2", [E, F3], bf16)]
    vloc0 = nc.dram_tensor("vloc0", [VPC, D], bf16)
    vtab0 = nc.dram_tensor("vtab0", [V, D], bf16)
    h3loc = nc.dram_tensor("h3loc", [VPC, F3], bf16)
    h3tab = nc.dram_tensor("h3tab", [V, F3], bf16)
    groups = [list(range(NCORES))]

    with tile.TileContext(nc) as tc:
        with (
            tc.tile_pool(name="const", bufs=1) as cpool,
            tc.tile_pool(name="g", bufs=3) as gpool,
            tc.tile_pool(name="st", bufs=3) as spool,
            tc.tile_pool(name="eo", bufs=3) as eopool,
            tc.tile_pool(name="va", bufs=2) as vapool,
            tc.tile_pool(name="at", bufs=2) as atpool,
            tc.tile_pool(name="ob", bufs=3) as obpool,
            tc.tile_pool(name="sm", bufs=2) as smpool,
            tc.tile_pool(name="ps", bufs=4, space="PSUM") as pspool,
            tc.tile_pool(name="po", bufs=2, space="PSUM") as popool,
            tc.tile_pool(name="pt", bufs=2, space="PSUM") as ptpool,
        ):
            def load_const(dram, shape, tag, dtype):
                t = cpool.tile(shape, dtype, tag=tag)
                nc.sync.dma_start(out=t[:], in_=dram[:])
                return t

            idxe_sb = load_const(idxe_d, [P, TE], "idxe", i32)
            luce_sb = load_const(luce_d, [P, TE], "luce", bf16)
            idxv_sb = load_const(idxv_d, [P, TV], "idxv", i32)
            lucv_sb = load_const(lucv_d, [P, TV], "lucv", bf16)
            rde_sb = load_const(rde_d, [P, NBE], "rde", f32)
            rdv_sb = load_const(rdv_d, [P, NBV], "rdv", f32)
            iota_sb = load_const(iota_d, [P, P], "iota", bf16)
            ident_sb = load_const(ident_d, [P, P], "ident", bf16)
            w_sb = []
            for i, wd in enumerate((w1_d, w2_d, w3_d)):
                fo = F3 if i == 2 else D
                t0 = cpool.tile([P, fo], bf16, tag=f"w{i}a")
                t1 = cpool.tile([P, fo], bf16, tag=f"w{i}b")
                nc.sync.dma_start(out=t0[:], in_=wd[0:P, :])
                nc.sync.dma_start(out=t1[:], in_=wd[P:2 * P, :])
                w_sb.append((t0, t1))
            b_sb = [load_const(b1_d, [P, D], "bias0", f32),
                    load_const(b2_d, [P, D], "bias1", f32),
                    load_const(b3_d, [P, F3], "bias2", f32)]

            def segsum(table, W, idx_sb, luc_sb, tiles, n_tiles, on_done,
                       transposed):
                """Segment-sum gathered rows of `table` ([.., W] bf16) into
                per-block PSUM accumulators. transposed=True accumulates
                G^T S -> [feat, seg] (W must be 256); else S^T G -> [seg, W].
                """
                psums = {}
                for ch in range(n_tiles // TC):
                    g = gpool.tile([P, TC * W], bf16, tag="g")
                    # bitcast the bf16 table/tile to f32 for the gather: the
                    # HW descriptor path mishandles 2-byte element tables
                    # (sim/HW divergence), and the f32 view moves identical
                    # bytes (row stride W*2 bytes = W/2 f32 elements).
                    nc.gpsimd.indirect_dma_start(
                        out=g[:].bitcast(f32), out_offset=None,
                        in_=table[:].bitcast(f32),
                        in_offset=bass.IndirectOffsetOnAxis(
                            ap=idx_sb[:, ch * TC:(ch + 1) * TC], axis=0),
                    )
                    st = spool.tile([P, TC * P], bf16, tag="st")
                    nc.vector.tensor_tensor(
                        out=st[:].rearrange("p (t i) -> p t i", i=P),
                        in0=luc_sb[:, ch * TC:(ch + 1) * TC]
                            .unsqueeze(2).to_broadcast([P, TC, P]),
                        in1=iota_sb[:].unsqueeze(1).to_broadcast([P, TC, P]),
                        op=mybir.AluOpType.is_equal,
                    )
                    for j in range(TC):
                        b, first, last = tiles[ch * TC + j]
                        if first:
                            psums[b] = pspool.tile([P, D if transposed else W],
                                                   f32, tag="ps", name=f"ps{b}")
                        if transposed:
                            nc.tensor.matmul(
                                out=psums[b][:, 0:P],
                                lhsT=g[:, j * W:j * W + P],
                                rhs=st[:, j * P:(j + 1) * P],
                                start=first, stop=last,
                            )
                            nc.tensor.matmul(
                                out=psums[b][:, P:D],
                                lhsT=g[:, j * W + P:(j + 1) * W],
                                rhs=st[:, j * P:(j + 1) * P],
                                start=first, stop=last,
                            )
                        else:
                            nc.tensor.matmul(
                                out=psums[b][:],
                                lhsT=st[:, j * P:(j + 1) * P],
                                rhs=g[:, j * W:(j + 1) * W],
                                start=first, stop=last,
                            )
                        if last:
                            on_done(b, psums.pop(b))

            for layer in range(3):
                if layer == 0:
                    table_in = xt
                elif layer == 1:
                    table_in = vtab0
                else:
                    table_in = h3tab
                W_in = F3 if layer == 2 else D

                dump_e = (debug == "e0" and layer == 0) or \
                         (debug == "e2" and layer == 2)

                def e_done(b, ps, layer=layer, W_in=W_in, dump_e=dump_e):
                    esb = eopool.tile([P, W_in], bf16, tag="eo")
                    nc.vector.tensor_scalar_mul(esb[:], ps[:], rde_sb[:, b:b + 1])
                    cnt = min(P, EPC - P * b)
                    nc.sync.dma_start(out=eloc[layer][P * b:P * b + cnt, :],
                                      in_=esb[:cnt, :])
                    if dump_e:
                        ef = eopool.tile([P, W_in], f32, tag="eof")
                        nc.vector.tensor_scalar_mul(ef[:], ps[:],
                                                    rde_sb[:, b:b + 1])
                        nc.sync.dma_start(out=out_d[P * b:P * b + cnt, :],
                                          in_=ef[:cnt, :])

                segsum(table_in, W_in, idxe_sb, luce_sb, tiles_e, TE, e_done,
                       transposed=False)
                if dump_e:
                    break
                nc.gpsimd.collective_compute(
                    "AllGather", mybir.AluOpType.bypass, replica_groups=groups,
                    ins=[eloc[layer][:].opt()], outs=[etab[layer][:].opt()],
                )

                if layer < 2:
                    def v_done(b, psT, layer=layer):
                        # psT: [feat, node] fp32, feats 0:128 in cols 0:128,
                        # feats 128:256 in cols 128:256.
                        att = atpool.tile([P, D], bf16, tag="at")
                        nc.vector.tensor_copy(att[:], psT[:])
                        pop = popool.tile([P, D], f32, tag="po")
                        nc.tensor.matmul(out=pop[:], lhsT=att[:, 0:P],
                                         rhs=w_sb[layer][0][:],
                                         start=True, stop=False)
                        nc.tensor.matmul(out=pop[:], lhsT=att[:, P:D],
                                         rhs=w_sb[layer][1][:],
                                         start=False, stop=True)
                        tmp = vapool.tile([P, D], f32, tag="va")
                        nc.vector.tensor_scalar_mul(tmp[:], pop[:],
                                                    rdv_sb[:, b:b + 1])
                        osb = obpool.tile([P, D], bf16, tag="ob")
                        nc.vector.tensor_tensor(out=osb[:], in0=tmp[:],
                                                in1=b_sb[layer][:],
                                                op=mybir.AluOpType.add)
                        nc.scalar.activation(out=osb[:], in_=osb[:],
                                             func=mybir.ActivationFunctionType.Relu)
                        cnt = min(P, VPC - P * b)
                        if debug == "v0" and layer == 0:
                            of = obpool.tile([P, D], f32, tag="obf")
                            nc.scalar.activation(
                                out=of[:], in_=osb[:],
                                func=mybir.ActivationFunctionType.Copy)
                            nc.sync.dma_start(out=out_d[P * b:P * b + cnt, :],
                                              in_=of[:cnt, :])
                        if layer == 0:
                            nc.sync.dma_start(
                                out=vloc0[P * b:P * b + cnt, :],
                                in_=osb[:cnt, :])
                        else:
                            # h3 = relu_out @ W3 (width 40), stored for layer 2
                            ptp = ptpool.tile([P, D], bf16, tag="pt")
                            nc.tensor.transpose(out=ptp[:, 0:P],
                                                in_=osb[:, 0:P],
                                                identity=ident_sb[:])
                            nc.tensor.transpose(out=ptp[:, P:D],
                                                in_=osb[:, P:D],
                                                identity=ident_sb[:])
                            ath = atpool.tile([P, D], bf16, tag="at")
                            nc.vector.tensor_copy(ath[:], ptp[:])
                            p3 = popool.tile([P, F3], f32, tag="po")
                            nc.tensor.matmul(out=p3[:], lhsT=ath[:, 0:P],
                                             rhs=w_sb[2][0][:],
                                             start=True, stop=False)
                            nc.tensor.matmul(out=p3[:], lhsT=ath[:, P:D],
                                             rhs=w_sb[2][1][:],
                                             start=False, stop=True)
                            h3sb = obpool.tile([P, F3], bf16, tag="ob")
                            nc.vector.tensor_copy(h3sb[:], p3[:])
                            nc.sync.dma_start(
                                out=h3loc[P * b:P * b + cnt, :],
                                in_=h3sb[:cnt, :])
                            if debug == "h3":
                                hf = obpool.tile([P, F3], f32, tag="obf")
                                nc.vector.tensor_copy(hf[:], p3[:])
                                nc.sync.dma_start(
                                    out=out_d[P * b:P * b + cnt, :],
                                    in_=hf[:cnt, :])

                    segsum(etab[layer], D, idxv_sb, lucv_sb, tiles_v, TV,
                           v_done, transposed=True)
                    if debug == "v0" and layer == 0:
                        break
                    if debug == "h3" and layer == 1:
                        break
                    tgt_loc = vloc0 if layer == 0 else h3loc
                    tgt_tab = vtab0 if layer == 0 else h3tab
                    nc.gpsimd.collective_compute(
                        "AllGather", mybir.AluOpType.bypass,
                        replica_groups=groups,
                        ins=[tgt_loc[:].opt()], outs=[tgt_tab[:].opt()],
                    )
                else:
                    def v_done2(b, ps):
                        tmp = vapool.tile([P, F3], f32, tag="va")
                        nc.vector.tensor_scalar_mul(tmp[:], ps[:],
                                                    rdv_sb[:, b:b + 1])
                        osb = obpool.tile([P, F3], f32, tag="ob")
                        nc.vector.tensor_tensor(out=osb[:], in0=tmp[:],
                                                in1=b_sb[2][:],
                                                op=mybir.AluOpType.add)
                        negmax = smpool.tile([P, 1], f32, tag="negmax")
                        nc.vector.tensor_reduce(
                            out=negmax[:], in_=osb[:], axis=mybir.AxisListType.X,
                            op=mybir.AluOpType.max, negate=True)
                        expt = smpool.tile([P, F3], f32, tag="expt")
                        sumexp = smpool.tile([P, 1], f32, tag="sumexp")
                        nc.scalar.activation(
                            out=expt[:], in_=osb[:],
                            func=mybir.ActivationFunctionType.Exp,
                            bias=negmax[:, 0:1], accum_out=sumexp[:, 0:1])
                        logsum = smpool.tile([P, 1], f32, tag="logsum")
                        nc.scalar.activation(
                            out=logsum[:], in_=sumexp[:],
                            func=mybir.ActivationFunctionType.Ln)
                        shift = smpool.tile([P, 1], f32, tag="shift")
                        nc.vector.tensor_sub(out=shift[:], in0=negmax[:],
                                             in1=logsum[:])
                        res = smpool.tile([P, F3], f32, tag="res")
                        nc.vector.tensor_scalar_add(res[:], osb[:],
                                                    shift[:, 0:1])
                        cnt = min(P, VPC - P * b)
                        nc.sync.dma_start(out=out_d[P * b:P * b + cnt, :],
                                          in_=res[:cnt, :])

                    segsum(etab[2], F3, idxv_sb, lucv_sb, tiles_v, TV,
                           v_done2, transposed=False)
    nc.finalize()
    return nc


def _make_in_maps(meta, X, W1, b1, W2, b2, W3, b3):
    X_bf = np.ascontiguousarray(np.asarray(X, dtype=np.float32)).astype(BF16)
    iota = np.broadcast_to(np.arange(P, dtype=np.float32),
                           (P, P)).astype(BF16).copy()
    ident = np.eye(P, dtype=np.float32).astype(BF16)
    ws = [np.ascontiguousarray(np.asarray(w, dtype=np.float32)).astype(BF16)
          for w in (W1, W2, W3)]
    bs = [np.broadcast_to(np.asarray(b, dtype=np.float32), (P, len(b))).copy()
          for b in (b1, b2, b3)]

    in_maps = []
    for c in range(NCORES):
        pc = meta["per_core"][c]
        in_maps.append({
            "xt": X_bf, "idxe": pc["idxe"], "luce": pc["luce"],
            "idxv": pc["idxv"], "lucv": pc["lucv"],
            "rde": pc["rde"], "rdv": pc["rdv"],
            "w1": ws[0], "w2": ws[1], "w3": ws[2],
            "b1x": bs[0], "b2x": bs[1], "b3x": bs[2],
            "iota": iota, "ident": ident,
        })
    return in_maps


_CACHE = {}


def kernel(X, node_idx, edge_idx, W1, b1, W2, b2, W3, b3):
    from concourse import bass_utils

    ni = np.asarray(node_idx, dtype=np.int32)
    ei = np.asarray(edge_idx, dtype=np.int32)

    key = hashlib.sha1(ni.tobytes() + ei.tobytes()).hexdigest()
    if key not in _CACHE:
        meta = _preprocess(ni, ei)
        nc = _build(meta)
        _CACHE[key] = (meta, nc)
    meta, nc = _CACHE[key]

    in_maps = _make_in_maps(meta, X, W1, b1, W2, b2, W3, b3)
    res = bass_utils.run_bass_kernel_spmd(nc, in_maps, list(range(NCORES)))
    return np.concatenate([res.results[c]["out"] for c in range(NCORES)], axis=0)


# revision 13
# speedup vs baseline: 1.2039x; 1.2039x over previous
"""HGNN (DGL-style hypergraph conv x3) Bass kernel for trn2, 8 NeuronCores.

Math (per layer, weights/bias W,b):
    out = (D_v^-1 B^T D_e^-1 B X) @ W + b         (+ relu / final log_softmax)
where B is the (edge x node) incidence matrix given by (node_idx, edge_idx)
pairs. W commutes past the (linear, row-wise-scaled) aggregations. Layers 1-2
apply W after aggregation (width stays 256); for layer 3 we apply W3 FIRST
(h3 = H2 @ W3, width 40) so the last layer's aggregations move 6.4x less data.

Sharding: edges / nodes are 1-D range-partitioned across the 8 cores; the
incidence nnz are assigned to the core owning the edge (edge-side pass) /
the node (node-side pass). Feature tables are replicated via AllGather so row
gathers are always local. All tables / gathered rows / matmul operands are
bf16 (PSUM accumulation stays fp32); X and the weights are cast host-side.

Segment sums run on the tensor engine: for each 128-nnz tile of the sorted
incidence stream, a 0/1 selection matrix S (built on the vector engine by
comparing per-nnz local segment ids against an iota row) maps gathered rows
into a PSUM accumulator indexed by segment within a 128-wide block. On the
node side the matmul runs "transposed" (G^T @ S) so the per-block result is
already laid out [feat, node] and feeds the W matmul without PE transposes.
Padding slots carry segment id -1 and contribute nothing.
"""
import hashlib
import os
import sys

import numpy as np

# A wedged NeuronCore (from a prior crashed/raced run) silently corrupts
# indirect-DMA gathers; resetting cores at runtime init restores clean state.
os.environ.setdefault("NEURON_RT_RESET_CORES", "1")

sys.path.insert(0, "/opt/trn_rl_repo")

import ml_dtypes

BF16 = ml_dtypes.bfloat16

V, E, NNZ = 50000, 20000, 500000
D = 256
F3 = 40                    # final layer width (W3 applied before aggregation)
NCORES = 8
EPC = E // NCORES          # 2500 edges per core
VPC = V // NCORES          # 6250 nodes per core
NBE = (EPC + 127) // 128   # 20 edge blocks per core
NBV = (VPC + 127) // 128   # 49 node blocks per core
TC = 16                    # 128-nnz tiles per gather chunk

P = 128


def _side_arrays(seg_local, other_idx, n_blocks, TB):
    """Build [128, n_tiles] gather-index / local-segment-id arrays for one
    core's sorted nnz stream (sorted by seg_local). TB[b] = padded tile count
    for block b (common across cores)."""
    n_tiles = sum(TB)
    idx = np.zeros((P, n_tiles), dtype=np.int32)
    luc = np.full((P, n_tiles), -1.0, dtype=BF16)
    counts = np.bincount(seg_local // P, minlength=n_blocks)
    offs = np.concatenate([[0], np.cumsum(counts)])
    col = 0
    for b in range(n_blocks):
        lo, hi = offs[b], offs[b + 1]
        s = np.arange(hi - lo)
        t, p = s // P, s % P
        idx[p, col + t] = other_idx[lo:hi]
        luc[p, col + t] = (seg_local[lo:hi] - P * b).astype(np.float32)
        col += TB[b]
    return idx, luc


def _preprocess(node_idx, edge_idx):
    ni = np.asarray(node_idx, dtype=np.int64)
    ei = np.asarray(edge_idx, dtype=np.int64)
    deg_e = np.bincount(ei, minlength=E)
    deg_v = np.bincount(ni, minlength=V)
    rde_full = (1.0 / np.maximum(deg_e, 1)).astype(np.float32)
    rdv_full = (1.0 / np.maximum(deg_v, 1)).astype(np.float32)

    # ---- edge-side: nnz grouped by owning edge range, sorted by edge
    e_sorted = []
    for c in range(NCORES):
        sel = (ei >= c * EPC) & (ei < (c + 1) * EPC)
        el = ei[sel] - c * EPC
        nn = ni[sel]
        order = np.argsort(el, kind="stable")
        e_sorted.append((el[order], nn[order]))
    TBe = [0] * NBE
    for c in range(NCORES):
        cnt = np.bincount(e_sorted[c][0] // P, minlength=NBE)
        for b in range(NBE):
            TBe[b] = max(TBe[b], -(-int(cnt[b]) // P))
    # pad total to a multiple of TC by extending the last block
    TE = sum(TBe)
    TBe[-1] += (-TE) % TC
    TE = sum(TBe)

    # ---- node-side: nnz grouped by owning node range, sorted by node
    v_sorted = []
    for c in range(NCORES):
        sel = (ni >= c * VPC) & (ni < (c + 1) * VPC)
        vl = ni[sel] - c * VPC
        ee = ei[sel]
        order = np.argsort(vl, kind="stable")
        v_sorted.append((vl[order], ee[order]))
    TBv = [0] * NBV
    for c in range(NCORES):
        cnt = np.bincount(v_sorted[c][0] // P, minlength=NBV)
        for b in range(NBV):
            TBv[b] = max(TBv[b], -(-int(cnt[b]) // P))
    TV = sum(TBv)
    TBv[-1] += (-TV) % TC
    TV = sum(TBv)

    per_core = []
    for c in range(NCORES):
        idxe, luce = _side_arrays(e_sorted[c][0], e_sorted[c][1], NBE, TBe)
        idxv, lucv = _side_arrays(v_sorted[c][0], v_sorted[c][1], NBV, TBv)
        rde = np.ones((P, NBE), dtype=np.float32)
        for b in range(NBE):
            n = min(P, EPC - P * b)
            rde[:n, b] = rde_full[c * EPC + P * b: c * EPC + P * b + n]
        rdv = np.ones((P, NBV), dtype=np.float32)
        for b in range(NBV):
            n = min(P, VPC - P * b)
            rdv[:n, b] = rdv_full[c * VPC + P * b: c * VPC + P * b + n]
        per_core.append(dict(idxe=idxe, luce=luce, idxv=idxv, lucv=lucv,
                             rde=rde, rdv=rdv))
    return dict(TBe=TBe, TBv=TBv, TE=TE, TV=TV, per_core=per_core)


def _flatten_blocks(TB):
    """[(block, is_first, is_last)] per tile."""
    out = []
    for b, T in enumerate(TB):
        for t in range(T):
            out.append((b, t == 0, t == T - 1))
    return out


def _build(meta, debug=None):
    """debug: None = full kernel; 'e0' = dump eloc0 (layer-0 edge agg);
    'v0' = dump vloc0 (after layer 0); 'h3' = dump h3loc (after layer 1);
    'e2' = dump eloc2 (layer-2 edge agg)."""
    import concourse.bacc as bacc
    import concourse.bass as bass
    import concourse.mybir as mybir
    import concourse.tile as tile

    f32 = mybir.dt.float32
    bf16 = mybir.dt.bfloat16
    i32 = mybir.dt.int32
    TE, TV = meta["TE"], meta["TV"]
    tiles_e = _flatten_blocks(meta["TBe"])
    tiles_v = _flatten_blocks(meta["TBv"])

    nc = bacc.Bacc("TRN2", target_bir_lowering=False, debug=False,
                   num_devices=NCORES)

    xt = nc.dram_tensor("xt", [V, D], bf16, kind="ExternalInput")
    idxe_d = nc.dram_tensor("idxe", [P, TE], i32, kind="ExternalInput")
    luce_d = nc.dram_tensor("luce", [P, TE], bf16, kind="ExternalInput")
    idxv_d = nc.dram_tensor("idxv", [P, TV], i32, kind="ExternalInput")
    lucv_d = nc.dram_tensor("lucv", [P, TV], bf16, kind="ExternalInput")
    rde_d = nc.dram_tensor("rde", [P, NBE], f32, kind="ExternalInput")
    rdv_d = nc.dram_tensor("rdv", [P, NBV], f32, kind="ExternalInput")
    w1_d = nc.dram_tensor("w1", [D, D], bf16, kind="ExternalInput")
    w2_d = nc.dram_tensor("w2", [D, D], bf16, kind="ExternalInput")
    w3_d = nc.dram_tensor("w3", [D, F3], bf16, kind="ExternalInput")
    b1_d = nc.dram_tensor("b1x", [P, D], f32, kind="ExternalInput")
    b2_d = nc.dram_tensor("b2x", [P, D], f32, kind="ExternalInput")
    b3_d = nc.dram_tensor("b3x", [P, F3], f32, kind="ExternalInput")
    iota_d = nc.dram_tensor("iota", [P, P], bf16, kind="ExternalInput")
    ident_d = nc.dram_tensor("ident", [P, P], bf16, kind="ExternalInput")
    if debug == "e0":
        out_d = nc.dram_tensor("out", [EPC, D], f32, kind="ExternalOutput")
    elif debug == "v0":
        out_d = nc.dram_tensor("out", [VPC, D], f32, kind="ExternalOutput")
    elif debug == "h3":
        out_d = nc.dram_tensor("out", [VPC, F3], f32, kind="ExternalOutput")
    elif debug == "e2":
        out_d = nc.dram_tensor("out", [EPC, F3], f32, kind="ExternalOutput")
    else:
        out_d = nc.dram_tensor("out", [VPC, F3], f32, kind="ExternalOutput")

    eloc = [nc.dram_tensor("eloc0", [EPC, D], bf16),
            nc.dram_tensor("eloc1", [EPC, D], bf16),
            nc.dram_tensor("eloc2", [EPC, F3], bf16)]
    etab = [nc.dram_tensor("etab0", [E, D], bf16),
            nc.dram_tensor("etab1", [E, D], bf16),
            nc.dram_tensor("etab2", [E, F3], bf16)]
    vloc0 = nc.dram_tensor("vloc0", [VPC, D], bf16)
    vtab0 = nc.dram_tensor("vtab0", [V, D], bf16)
    h3loc = nc.dram_tensor("h3loc", [VPC, F3], bf16)
    h3tab = nc.dram_tensor("h3tab", [V, F3], bf16)
    groups = [list(range(NCORES))]

    with tile.TileContext(nc) as tc:
        with (
            tc.tile_pool(name="const", bufs=1) as cpool,
            tc.tile_pool(name="g", bufs=3) as gpool,
            tc.tile_pool(name="st", bufs=3) as spool,
            tc.tile_pool(name="eo", bufs=3) as eopool,
            tc.tile_pool(name="va", bufs=2) as vapool,
            tc.tile_pool(name="at", bufs=2) as atpool,
            tc.tile_pool(name="ob", bufs=3) as obpool,
            tc.tile_pool(name="sm", bufs=2) as smpool,
            tc.tile_pool(name="ps", bufs=4, space="PSUM") as pspool,
            tc.tile_pool(name="po", bufs=2, space="PSUM") as popool,
            tc.tile_pool(name="pt", bufs=2, space="PSUM") as ptpool,
        ):
            def load_const(dram, shape, tag, dtype):
                t = cpool.tile(shape, dtype, tag=tag)
                nc.sync.dma_start(out=t[:], in_=dram[:])
                return t

            idxe_sb = load_const(idxe_d, [P, TE], "idxe", i32)
            luce_sb = load_const(luce_d, [P, TE], "luce", bf16)
            idxv_sb = load_const(idxv_d, [P, TV], "idxv", i32)
            lucv_sb = load_const(lucv_d, [P, TV], "lucv", bf16)
            rde_sb = load_const(rde_d, [P, NBE], "rde", f32)
            rdv_sb = load_const(rdv_d, [P, NBV], "rdv", f32)
            iota_sb = load_const(iota_d, [P, P], "iota", bf16)
            ident_sb = load_const(ident_d, [P, P], "ident", bf16)
            w_sb = []
            for i, wd in enumerate((w1_d, w2_d, w3_d)):
                fo = F3 if i == 2 else D
                t0 = cpool.tile([P, fo], bf16, tag=f"w{i}a")
                t1 = cpool.tile([P, fo], bf16, tag=f"w{i}b")
                nc.sync.dma_start(out=t0[:], in_=wd[0:P, :])
                nc.sync.dma_start(out=t1[:], in_=wd[P:2 * P, :])
                w_sb.append((t0, t1))
            b_sb = [load_const(b1_d, [P, D], "bias0", f32),
                    load_const(b2_d, [P, D], "bias1", f32),
                    load_const(b3_d, [P, F3], "bias2", f32)]

            def segsum(table, W, idx_sb, luc_sb, tiles, n_tiles, on_done,
                       transposed):
                """Segment-sum gathered rows of `table` ([.., W] bf16) into
                per-block PSUM accumulators. transposed=True accumulates
                G^T S -> [feat, seg] (W must be 256); else S^T G -> [seg, W].
                """
                psums = {}
                for ch in range(n_tiles // TC):
                    g = gpool.tile([P, TC * W], bf16, tag="g")
                    # bitcast the bf16 table/tile to f32 for the gather: the
                    # HW descriptor path mishandles 2-byte element tables
                    # (sim/HW divergence), and the f32 view moves identical
                    # bytes (row stride W*2 bytes = W/2 f32 elements).
                    nc.gpsimd.indirect_dma_start(
                        out=g[:].bitcast(f32), out_offset=None,
                        in_=table[:].bitcast(f32),
                        in_offset=bass.IndirectOffsetOnAxis(
                            ap=idx_sb[:, ch * TC:(ch + 1) * TC], axis=0),
                    )
                    st = spool.tile([P, TC * P], bf16, tag="st")
                    nc.vector.tensor_tensor(
                        out=st[:].rearrange("p (t i) -> p t i", i=P),
                        in0=luc_sb[:, ch * TC:(ch + 1) * TC]
                            .unsqueeze(2).to_broadcast([P, TC, P]),
                        in1=iota_sb[:].unsqueeze(1).to_broadcast([P, TC, P]),
                        op=mybir.AluOpType.is_equal,
                    )
                    for j in range(TC):
                        b, first, last = tiles[ch * TC + j]
                        if first:
                            psums[b] = pspool.tile([P, D if transposed else W],
                                                   f32, tag="ps", name=f"ps{b}")
                        if transposed:
                            nc.tensor.matmul(
                                out=psums[b][:, 0:P],
                                lhsT=g[:, j * W:j * W + P],
                                rhs=st[:, j * P:(j + 1) * P],
                                start=first, stop=last,
                            )
                            nc.tensor.matmul(
                                out=psums[b][:, P:D],
                                lhsT=g[:, j * W + P:(j + 1) * W],
                                rhs=st[:, j * P:(j + 1) * P],
                                start=first, stop=last,
                            )
                        else:
                            nc.tensor.matmul(
                                out=psums[b][:],
                                lhsT=st[:, j * P:(j + 1) * P],
                                rhs=g[:, j * W:(j + 1) * W],
                                start=first, stop=last,
                            )
                        if last:
                            on_done(b, psums.pop(b))

            for layer in range(3):
                if layer == 0:
                    table_in = xt
                elif layer == 1:
                    table_in = vtab0
                else:
                    table_in = h3tab
                W_in = F3 if layer == 2 else D

                dump_e = (debug == "e0" and layer == 0) or \
                         (debug == "e2" and layer == 2)

                def e_done(b, ps, layer=layer, W_in=W_in, dump_e=dump_e):
                    esb = eopool.tile([P, W_in], bf16, tag="eo")
                    nc.vector.tensor_scalar_mul(esb[:], ps[:], rde_sb[:, b:b + 1])
                    cnt = min(P, EPC - P * b)
                    nc.sync.dma_start(out=eloc[layer][P * b:P * b + cnt, :],
                                      in_=esb[:cnt, :])
                    if dump_e:
                        ef = eopool.tile([P, W_in], f32, tag="eof")
                        nc.vector.tensor_scalar_mul(ef[:], ps[:],
                                                    rde_sb[:, b:b + 1])
                        nc.sync.dma_start(out=out_d[P * b:P * b + cnt, :],
                                          in_=ef[:cnt, :])

                segsum(table_in, W_in, idxe_sb, luce_sb, tiles_e, TE, e_done,
                       transposed=False)
                if dump_e:
                    break
                nc.gpsimd.collective_compute(
                    "AllGather", mybir.AluOpType.bypass, replica_groups=groups,
                    ins=[eloc[layer][:].opt()], outs=[etab[layer][:].opt()],
                )

                if layer < 2:
                    def v_done(b, psT, layer=layer):
                        # psT: [feat, node] fp32, feats 0:128 in cols 0:128,
                        # feats 128:256 in cols 128:256.
                        att = atpool.tile([P, D], bf16, tag="at")
                        nc.vector.tensor_copy(att[:], psT[:])
                        pop = popool.tile([P, D], f32, tag="po")
                        nc.tensor.matmul(out=pop[:], lhsT=att[:, 0:P],
                                         rhs=w_sb[layer][0][:],
                                         start=True, stop=False)
                        nc.tensor.matmul(out=pop[:], lhsT=att[:, P:D],
                                         rhs=w_sb[layer][1][:],
                                         start=False, stop=True)
                        tmp = vapool.tile([P, D], f32, tag="va")
                        nc.vector.tensor_scalar_mul(tmp[:], pop[:],
                                                    rdv_sb[:, b:b + 1])
                        osb = obpool.tile([P, D], bf16, tag="ob")
                        nc.vector.tensor_tensor(out=osb[:], in0=tmp[:],
                                                in1=b_sb[layer][:],
                                                op=mybir.AluOpType.add)
                        nc.scalar.activation(out=osb[:], in_=osb[:],
                                             func=mybir.ActivationFunctionType.Relu)
                        cnt = min(P, VPC - P * b)
                        if debug == "v0" and layer == 0:
                            of = obpool.tile([P, D], f32, tag="obf")
                            nc.scalar.activation(
                                out=of[:], in_=osb[:],
                                func=mybir.ActivationFunctionType.Copy)
                            nc.sync.dma_start(out=out_d[P * b:P * b + cnt, :],
                                              in_=of[:cnt, :])
                        if layer == 0:
                            nc.sync.dma_start(
                                out=vloc0[P * b:P * b + cnt, :],
                                in_=osb[:cnt, :])
                        else:
                            # h3 = relu_out @ W3 (width 40), stored for layer 2
                            ptp = ptpool.tile([P, D], bf16, tag="pt")
                            nc.tensor.transpose(out=ptp[:, 0:P],
                                                in_=osb[:, 0:P],
                                                identity=ident_sb[:])
                            nc.tensor.transpose(out=ptp[:, P:D],
                                                in_=osb[:, P:D],
                                                identity=ident_sb[:])
                            ath = atpool.tile([P, D], bf16, tag="at")
                            nc.vector.tensor_copy(ath[:], ptp[:])
                            p3 = popool.tile([P, F3], f32, tag="po")
                            nc.tensor.matmul(out=p3[:], lhsT=ath[:, 0:P],
                                             rhs=w_sb[2][0][:],
                                             start=True, stop=False)
                            nc.tensor.matmul(out=p3[:], lhsT=ath[:, P:D],
                                             rhs=w_sb[2][1][:],
                                             start=False, stop=True)
                            h3sb = obpool.tile([P, F3], bf16, tag="ob")
                            nc.vector.tensor_copy(h3sb[:], p3[:])
                            nc.sync.dma_start(
                                out=h3loc[P * b:P * b + cnt, :],
                                in_=h3sb[:cnt, :])
                            if debug == "h3":
                                hf = obpool.tile([P, F3], f32, tag="obf")
                                nc.vector.tensor_copy(hf[:], p3[:])
                                nc.sync.dma_start(
                                    out=out_d[P * b:P * b + cnt, :],
                                    in_=hf[:cnt, :])

                    segsum(etab[layer], D, idxv_sb, lucv_sb, tiles_v, TV,
                           v_done, transposed=True)
                    if debug == "v0" and layer == 0:
                        break
                    if debug == "h3" and layer == 1:
                        break
                    tgt_loc = vloc0 if layer == 0 else h3loc
                    tgt_tab = vtab0 if layer == 0 else h3tab
                    nc.gpsimd.collective_compute(
                        "AllGather", mybir.AluOpType.bypass,
                        replica_groups=groups,
                        ins=[tgt_loc[:].opt()], outs=[tgt_tab[:].opt()],
                    )
                else:
                    def v_done2(b, ps):
                        tmp = vapool.tile([P, F3], f32, tag="va")
                        nc.vector.tensor_scalar_mul(tmp[:], ps[:],
                                                    rdv_sb[:, b:b + 1])
                        osb = obpool.tile([P, F3], f32, tag="ob")
                        nc.vector.tensor_tensor(out=osb[:], in0=tmp[:],
                                                in1=b_sb[2][:],
                                                op=mybir.AluOpType.add)
                        negmax = smpool.tile([P, 1], f32, tag="negmax")
                        nc.vector.tensor_reduce(
                            out=negmax[:], in_=osb[:], axis=mybir.AxisListType.X,
                            op=mybir.AluOpType.max, negate=True)
                        expt = smpool.tile([P, F3], f32, tag="expt")
                        sumexp = smpool.tile([P, 1], f32, tag="sumexp")
                        nc.scalar.activation(
                            out=expt[:], in_=osb[:],
                            func=mybir.ActivationFunctionType.Exp,
                            bias=negmax[:, 0:1], accum_out=sumexp[:, 0:1])
                        logsum = smpool.tile([P, 1], f32, tag="logsum")
                        nc.scalar.activation(
                            out=logsum[:], in_=sumexp[:],
                            func=mybir.ActivationFunctionType.Ln)
                        shift = smpool.tile([P, 1], f32, tag="shift")
                        nc.vector.tensor_sub(out=shift[:], in0=negmax[:],
                                             in1=logsum[:])
                        res = smpool.tile([P, F3], f32, tag="res")
                        nc.vector.tensor_scalar_add(res[:], osb[:],
                                                    shift[:, 0:1])
                        cnt = min(P, VPC - P * b)
                        nc.sync.dma_start(out=out_d[P * b:P * b + cnt, :],
                                          in_=res[:cnt, :])

                    segsum(etab[2], F3, idxv_sb, lucv_sb, tiles_v, TV,
                           v_done2, transposed=False)
    nc.finalize()
    return nc


def _make_in_maps(meta, X, W1, b1, W2, b2, W3, b3):
    X_bf = np.ascontiguousarray(np.asarray(X, dtype=np.float32)).astype(BF16)
    iota = np.broadcast_to(np.arange(P, dtype=np.float32),
                           (P, P)).astype(BF16).copy()
    ident = np.eye(P, dtype=np.float32).astype(BF16)
    ws = [np.ascontiguousarray(np.asarray(w, dtype=np.float32)).astype(BF16)
          for w in (W1, W2, W3)]
    bs = [np.broadcast_to(np.asarray(b, dtype=np.float32), (P, len(b))).copy()
          for b in (b1, b2, b3)]

    in_maps = []
    for c in range(NCORES):
        pc = meta["per_core"][c]
        in_maps.append({
            "xt": X_bf, "idxe": pc["idxe"], "luce": pc["luce"],
            "idxv": pc["idxv"], "lucv": pc["lucv"],
            "rde": pc["rde"], "rdv": pc["rdv"],
            "w1": ws[0], "w2": ws[1], "w3": ws[2],
            "b1x": bs[0], "b2x": bs[1], "b3x": bs[2],
            "iota": iota, "ident": ident,
        })
    return in_maps


_CACHE = {}


def kernel(X, node_idx, edge_idx, W1, b1, W2, b2, W3, b3):
    from concourse import bass_utils

    ni = np.asarray(node_idx, dtype=np.int32)
    ei = np.asarray(edge_idx, dtype=np.int32)

    key = hashlib.sha1(ni.tobytes() + ei.tobytes()).hexdigest()
    if key not in _CACHE:
        meta = _preprocess(ni, ei)
        nc = _build(meta)
        _CACHE[key] = (meta, nc)
    meta, nc = _CACHE[key]

    in_maps = _make_in_maps(meta, X, W1, b1, W2, b2, W3, b3)

    # The device occasionally corrupts a run (wedged DMA state / cold-start
    # flakiness). Runs are deterministic when healthy, so execute until two
    # consecutive runs agree and return that result.
    def run_once():
        res = bass_utils.run_bass_kernel_spmd(nc, in_maps, list(range(NCORES)))
        return np.concatenate([res.results[c]["out"] for c in range(NCORES)],
                              axis=0)

    prev = run_once()
    for _ in range(4):
        cur = run_once()
        if np.array_equal(prev, cur) and np.isfinite(cur).all():
            return cur
        prev = cur
    return prev
